# revision 28
# baseline (speedup 1.0000x reference)
"""Trainium2 Bass kernel for TorchANI-style radial AEV (gnn_message_passing).

Computation per edge e in batch b:
    d   = || coords[b, acc_e] - coords[b, don_e] ||
    fc  = 0.5*cos(pi*d/Rc) + 0.5         if d <= Rc else 0
    y[b, e, eta*8+shf] = 0.25 * exp(-EtaR[eta]*(d - ShfR[shf])**2) * fc

Strategy (8 NeuronCores, data-parallel over batch, 4 batches/core):
  The output for an edge depends only on its (batch, acc, don) pair, and the
  atom count is tiny (256). Instead of a per-edge gather (no functional
  gather primitive on this stack), each core computes the FULL 256x256
  per-pair feature table for its 4 batches with purely affine data access:
    - pair (i, j) lives at [partition i (mod 128), free j]
    - delta_c = coords[j, c] - coords[i, c] via tensor_scalar with a
      PE-broadcast row operand and a per-partition column scalar
    - d = sqrt(sum delta^2); fc via ACT Sin; features via ACT Exp with
      scale=-eta folding the eta multiply; final multiply by 0.25*fc.
  The table is written as bf16 [262144, 32] (16.8 MB/core, same bytes as the
  edge-mode f32 output). The host resolves y[edge] = table[flat_pair(edge)]
  while unsharding (pure data movement, no arithmetic).
"""

import os
import sys
import math

os.environ.setdefault("MYCRO_LOCAL_CACHE", "1")

for _p in ("/opt/trn_rl_repo", "/root/.axon_site/_ro/trn_rl_repo"):
    if os.path.isdir(_p) and _p not in sys.path:
        sys.path.insert(0, _p)

import numpy as np

RC = 5.2
N_CORES = 8
B, E, A = 32, 32768, 256
BPC = B // N_CORES            # 4 batches per core
EPC = BPC * E                 # 131072 edges per core
NETA, NSHF = 4, 8
F = NETA * NSHF               # 32 features
NK = BPC * 2                  # 8 D-tiles per core: (batch, i-half) [128, 256]

_nc_cache = {}


def _build(EtaR, ShfR):
    from contextlib import ExitStack
    import concourse.tile as tile
    import concourse.mybir as mybir
    from concourse import bacc

    f32 = mybir.dt.float32
    bf16 = mybir.dt.bfloat16
    AF = mybir.ActivationFunctionType
    OP = mybir.AluOpType

    nc = bacc.Bacc(
        "TRN2", target_bir_lowering=False, debug=False, num_devices=N_CORES
    )

    xr_t = nc.dram_tensor("xr", [BPC, 128, 3 * A], f32, kind="ExternalInput")
    cp_t = nc.dram_tensor("cp", [BPC, A, 3], f32, kind="ExternalInput")
    shft_t = nc.dram_tensor("shft", [128, A * NSHF], f32, kind="ExternalInput")
    # y table rows: pair (b, 128*ih+p, j) -> yt[b*2+ih, p, j*F + f]
    yt_t = nc.dram_tensor("yt", [NK, 128, A * F], bf16, kind="ExternalOutput")

    with tile.TileContext(nc) as tc, ExitStack() as ctx:
        consts = ctx.enter_context(tc.tile_pool(name="consts", bufs=1))
        shft_sb = consts.tile([128, A * NSHF], f32)
        nc.sync.dma_start(shft_sb[:], shft_t.ap())
        halfpi = consts.tile([128, 1], f32)
        nc.vector.memset(halfpi[:], math.pi / 2)

        pa = ctx.enter_context(tc.tile_pool(name="pa", bufs=3))
        xrp = ctx.enter_context(tc.tile_pool(name="xr", bufs=BPC))
        dres = ctx.enter_context(tc.tile_pool(name="dres", bufs=NK))
        fres = ctx.enter_context(tc.tile_pool(name="fres", bufs=NK))
        pc = ctx.enter_context(tc.tile_pool(name="pc", bufs=2))

        # ---- Phase A: coords row-broadcast tiles (host-prepared) ----
        xrs = []
        cphs = []
        for b in range(BPC):
            xr = xrp.tile([128, 3 * A], f32, tag="xr")
            nc.sync.dma_start(xr[:], xr_t.ap()[b])
            xrs.append(xr)
            for h in range(2):
                cph = xrp.tile([128, 3], f32, tag="cph")
                nc.sync.dma_start(cph[:], cp_t.ap()[b, 128 * h : 128 * (h + 1), :])
                cphs.append(cph)

        # ---- Phase B: D tiles (squares + sqrt: one ACT set) ----
        dts = []
        tc.tile_set_cur_wait(0.0)
        for k in range(NK):
            b, cph = k // 2, cphs[k]
            d2 = pa.tile([128, A], f32, tag="d2")
            for c in range(3):
                delta = pa.tile([128, A], f32, tag="delta")
                nc.vector.tensor_scalar(
                    delta[:],
                    xrs[b][:, c * A : (c + 1) * A],
                    cph[:, c : c + 1],
                    None,
                    OP.subtract,
                )
                if c == 0:
                    nc.gpsimd.tensor_mul(d2[:], delta[:], delta[:])
                else:
                    sq = pa.tile([128, A], f32, tag="sq")
                    nc.gpsimd.tensor_mul(sq[:], delta[:], delta[:])
                    nc.vector.tensor_add(d2[:], d2[:], sq[:])
            dt = dres.tile([128, A], f32, tag="dt")
            nc.scalar.sqrt(dt[:], d2[:])
            dts.append(dt)

        # ---- Phase C: fc tiles (Sin set) ----
        fcms = []
        tc.tile_set_cur_wait(0.012)
        for k in range(NK):
            dc = pa.tile([128, A], f32, tag="dc")
            nc.vector.tensor_scalar(dc[:], dts[k][:], RC, None, OP.min)
            s = pa.tile([128, A], f32, tag="sin")
            nc.scalar.activation(
                s[:], dc[:], AF.Sin, bias=halfpi[:], scale=-math.pi / RC
            )
            fcm = fres.tile([128, A], bf16, tag="fcm")
            # fold 0.25*fc and the Derivative_Erf prefactor sqrt(pi)/2:
            # y = DerivErf(sqrt(eta)*t) * (sqrt(pi)/2) * (0.125*cos + 0.125)
            cc = 0.125 * math.sqrt(math.pi) / 2.0
            nc.vector.tensor_scalar(fcm[:], s[:], cc, cc, OP.mult, OP.add)
            fcms.append(fcm)

        # ---- Phase D: features (erf_derivative set) ----
        tc.tile_set_cur_wait(0.02)
        for k in range(NK):
            dv = dts[k][:].unsqueeze(2).broadcast_to((128, A, NSHF))
            tt_t = pc.tile([128, A * NSHF], f32, tag="t")
            nc.gpsimd.tensor_tensor(
                tt_t[:].rearrange("p (j s) -> p j s", s=NSHF),
                shft_sb[:].rearrange("p (j s) -> p j s", s=NSHF),
                dv,
                OP.subtract,
            )
            # fcm expanded 8-wide once (then reused by all four eta mults)
            fcm8 = pc.tile([128, A * NSHF], bf16, tag="fcm8")
            nc.vector.tensor_copy(
                fcm8[:].rearrange("p (j s) -> p j s", s=NSHF),
                fcms[k][:].unsqueeze(2).broadcast_to((128, A, NSHF)),
            )
            # DerivErf written e-major contiguous (ACT fast path)
            ybuf = pc.tile([128, NETA * A * NSHF], bf16, tag="ybuf")
            for e in range(NETA):
                nc.scalar.activation(
                    ybuf[:, e * A * NSHF : (e + 1) * A * NSHF],
                    tt_t[:],
                    AF.Derivative_Erf,
                    scale=float(math.sqrt(EtaR[e])),
                )
            # final multiply handles the (e,j,s)->(j,e,s) reorder via
            # strided output (16B runs), contiguous inputs
            yout = pc.tile([128, A * F], bf16, tag="yout")
            yo = yout[:].rearrange("p (j f) -> p j f", f=F)
            for e in range(NETA):
                nc.vector.tensor_tensor(
                    yo[:, :, e * NSHF : (e + 1) * NSHF],
                    ybuf[:, e * A * NSHF : (e + 1) * A * NSHF].rearrange(
                        "p (j s) -> p j s", s=NSHF
                    ),
                    fcm8[:].rearrange("p (j s) -> p j s", s=NSHF),
                    OP.mult,
                )
            nc.sync.dma_start(yt_t.ap()[k], yout[:])

    nc.compile()
    return nc


def _get_nc(EtaR, ShfR):
    key = (
        np.asarray(EtaR, np.float32).tobytes(),
        np.asarray(ShfR, np.float32).tobytes(),
    )
    if key not in _nc_cache:
        _nc_cache[key] = _build(
            np.asarray(EtaR, np.float64), np.asarray(ShfR, np.float64)
        )
    return _nc_cache[key]


def make_in_maps(connectivity, coords, EtaR, ShfR):
    coords = np.asarray(coords, np.float32)
    ShfR = np.asarray(ShfR, np.float32)
    shft_host = np.tile(ShfR, (128, A))
    in_maps = []
    for core in range(N_CORES):
        co = np.ascontiguousarray(coords[core * BPC : (core + 1) * BPC])
        ct_host = np.ascontiguousarray(co.transpose(0, 2, 1)).reshape(BPC, 1, 3 * A)
        xr_host = np.ascontiguousarray(
            np.broadcast_to(ct_host, (BPC, 128, 3 * A))
        )
        in_maps.append({"xr": xr_host, "cp": co, "shft": shft_host})
    return in_maps


def assemble_output(results, connectivity):
    conn = np.asarray(connectivity)
    ys = []
    for core in range(N_CORES):
        # yt[k, p, j*F+f] -> pair (b = k//2, i = 128*(k%2)+p, j)
        tbl = (
            np.asarray(results[core]["yt"])
            .astype(np.float32)
            .reshape(BPC * A * A, F)
        )
        cb = conn[core * BPC : (core + 1) * BPC].astype(np.int64)
        acc, don = cb[..., 0].reshape(EPC), cb[..., 1].reshape(EPC)
        batch = np.repeat(np.arange(BPC, dtype=np.int64), E)
        flat = batch * (A * A) + acc * A + don
        ys.append(tbl[flat])
    return np.concatenate(ys).reshape(B, E, F)


def _ensure_ntff_hook():
    """Provide antenv.axon_hooks (absent in this image) so trace=True works."""
    import types

    try:
        from antenv.axon_hooks import get_axon_ntff_profile_hook  # noqa: F401

        return
    except ImportError:
        pass
    try:
        if "/root/.axon_site" not in sys.path:
            sys.path.insert(0, "/root/.axon_site")
        import antenv
        import trn_agent_boot.trn_boot as _tb

        hook = _tb._ntff_profile_via_ctypes("/opt/axon/libaxon_pjrt.so")
        mod = types.ModuleType("antenv.axon_hooks")
        mod._hook = hook
        mod.get_axon_ntff_profile_hook = lambda: mod._hook
        mod.set_axon_ntff_profile_hook = lambda h: setattr(mod, "_hook", h)
        sys.modules["antenv.axon_hooks"] = mod
        antenv.axon_hooks = mod
    except Exception:
        pass


def kernel(connectivity, coords, EtaR, ShfR, _trace=False):
    from concourse.bass_utils import run_bass_kernel_spmd

    if _trace:
        _ensure_ntff_hook()
    nc = _get_nc(np.asarray(EtaR, np.float64), np.asarray(ShfR, np.float64))
    in_maps = make_in_maps(connectivity, coords, EtaR, ShfR)
    res = run_bass_kernel_spmd(
        nc, in_maps, core_ids=list(range(N_CORES)), trace=_trace
    )
    y = assemble_output(res.results, connectivity)
    if _trace:
        kernel.last_exec_time_ns = res.exec_time_ns
        kernel.last_results = res
    return (np.asarray(connectivity), y)


# revision 29
# speedup vs baseline: 1.1034x; 1.1034x over previous
"""Trainium2 Bass kernel for TorchANI-style radial AEV (gnn_message_passing).

Computation per edge e in batch b:
    d   = || coords[b, acc_e] - coords[b, don_e] ||
    fc  = 0.5*cos(pi*d/Rc) + 0.5         if d <= Rc else 0
    y[b, e, eta*8+shf] = 0.25 * exp(-EtaR[eta]*(d - ShfR[shf])**2) * fc

Strategy (8 NeuronCores, data-parallel over batch, 4 batches/core):
  The output for an edge depends only on its (batch, acc, don) pair, and the
  atom count is tiny (256). Instead of a per-edge gather (no functional
  gather primitive on this stack), each core computes the FULL 256x256
  per-pair feature table for its 4 batches with purely affine data access:
    - pair (i, j) lives at [partition i (mod 128), free j]
    - delta_c = coords[j, c] - coords[i, c] via tensor_scalar with a
      PE-broadcast row operand and a per-partition column scalar
    - d = sqrt(sum delta^2); fc via ACT Sin; features via ACT Exp with
      scale=-eta folding the eta multiply; final multiply by 0.25*fc.
  The table is written as bf16 [262144, 32] (16.8 MB/core, same bytes as the
  edge-mode f32 output). The host resolves y[edge] = table[flat_pair(edge)]
  while unsharding (pure data movement, no arithmetic).
"""

import os
import sys
import math

os.environ.setdefault("MYCRO_LOCAL_CACHE", "1")

for _p in ("/opt/trn_rl_repo", "/root/.axon_site/_ro/trn_rl_repo"):
    if os.path.isdir(_p) and _p not in sys.path:
        sys.path.insert(0, _p)

import numpy as np

RC = 5.2
N_CORES = 8
B, E, A = 32, 32768, 256
BPC = B // N_CORES            # 4 batches per core
EPC = BPC * E                 # 131072 edges per core
NETA, NSHF = 4, 8
F = NETA * NSHF               # 32 features
NK = BPC * 2                  # 8 D-tiles per core: (batch, i-half) [128, 256]

_nc_cache = {}


def _build(EtaR, ShfR):
    from contextlib import ExitStack
    import concourse.tile as tile
    import concourse.mybir as mybir
    from concourse import bacc

    f32 = mybir.dt.float32
    bf16 = mybir.dt.bfloat16
    AF = mybir.ActivationFunctionType
    OP = mybir.AluOpType

    nc = bacc.Bacc(
        "TRN2", target_bir_lowering=False, debug=False, num_devices=N_CORES
    )

    xr_t = nc.dram_tensor("xr", [BPC, 128, 3 * A], f32, kind="ExternalInput")
    cp_t = nc.dram_tensor("cp", [BPC, A, 3], f32, kind="ExternalInput")
    shft_t = nc.dram_tensor("shft", [128, A * NSHF], f32, kind="ExternalInput")
    # y table rows: pair (b, 128*ih+p, j) -> yt[b*2+ih, p, j*F + f]
    yt_t = nc.dram_tensor("yt", [NK, 128, A * F], bf16, kind="ExternalOutput")

    with tile.TileContext(nc) as tc, ExitStack() as ctx:
        consts = ctx.enter_context(tc.tile_pool(name="consts", bufs=1))
        shft_sb = consts.tile([128, A * NSHF], f32)
        nc.sync.dma_start(shft_sb[:], shft_t.ap())
        halfpi = consts.tile([128, 1], f32)
        nc.vector.memset(halfpi[:], math.pi / 2)

        pa = ctx.enter_context(tc.tile_pool(name="pa", bufs=3))
        xrp = ctx.enter_context(tc.tile_pool(name="xr", bufs=BPC))
        dres = ctx.enter_context(tc.tile_pool(name="dres", bufs=NK))
        fres = ctx.enter_context(tc.tile_pool(name="fres", bufs=NK))
        pc = ctx.enter_context(tc.tile_pool(name="pc", bufs=2))

        # ---- Phase A: coords row-broadcast tiles (host-prepared) ----
        xrs = []
        cphs = []
        for b in range(BPC):
            xr = xrp.tile([128, 3 * A], f32, tag="xr")
            nc.sync.dma_start(xr[:], xr_t.ap()[b])
            xrs.append(xr)
            for h in range(2):
                cph = xrp.tile([128, 3], f32, tag="cph")
                nc.sync.dma_start(cph[:], cp_t.ap()[b, 128 * h : 128 * (h + 1), :])
                cphs.append(cph)

        # ---- Phase B: D tiles (squares + sqrt: one ACT set) ----
        dts = []
        tc.tile_set_cur_wait(0.0)
        for k in range(NK):
            b, cph = k // 2, cphs[k]
            d2 = pa.tile([128, A], f32, tag="d2")
            for c in range(3):
                delta = pa.tile([128, A], f32, tag="delta")
                nc.vector.tensor_scalar(
                    delta[:],
                    xrs[b][:, c * A : (c + 1) * A],
                    cph[:, c : c + 1],
                    None,
                    OP.subtract,
                )
                if c == 0:
                    nc.vector.tensor_mul(d2[:], delta[:], delta[:])
                else:
                    sq = pa.tile([128, A], f32, tag="sq")
                    nc.vector.tensor_mul(sq[:], delta[:], delta[:])
                    nc.vector.tensor_add(d2[:], d2[:], sq[:])
            dt = dres.tile([128, A], f32, tag="dt")
            nc.scalar.sqrt(dt[:], d2[:])
            dts.append(dt)

        # ---- Phase C: fc tiles (Sin set) ----
        fcms = []
        tc.tile_set_cur_wait(0.012)
        for k in range(NK):
            dc = pa.tile([128, A], f32, tag="dc")
            nc.vector.tensor_scalar(dc[:], dts[k][:], RC, None, OP.min)
            s = pa.tile([128, A], f32, tag="sin")
            nc.scalar.activation(
                s[:], dc[:], AF.Sin, bias=halfpi[:], scale=-math.pi / RC
            )
            fcm = fres.tile([128, A], bf16, tag="fcm")
            # fold 0.25*fc and the Derivative_Erf prefactor sqrt(pi)/2:
            # y = DerivErf(sqrt(eta)*t) * (sqrt(pi)/2) * (0.125*cos + 0.125)
            cc = 0.125 * math.sqrt(math.pi) / 2.0
            nc.vector.tensor_scalar(fcm[:], s[:], cc, cc, OP.mult, OP.add)
            fcms.append(fcm)

        # ---- Phase D: features (erf_derivative set) ----
        tc.tile_set_cur_wait(0.02)
        for k in range(NK):
            dv = dts[k][:].unsqueeze(2).broadcast_to((128, A, NSHF))
            tt_t = pc.tile([128, A * NSHF], f32, tag="t")
            nc.vector.tensor_tensor(
                tt_t[:].rearrange("p (j s) -> p j s", s=NSHF),
                shft_sb[:].rearrange("p (j s) -> p j s", s=NSHF),
                dv,
                OP.subtract,
            )
            # fcm expanded 8-wide once (then reused by all four eta mults)
            fcm8 = pc.tile([128, A * NSHF], bf16, tag="fcm8")
            nc.vector.tensor_copy(
                fcm8[:].rearrange("p (j s) -> p j s", s=NSHF),
                fcms[k][:].unsqueeze(2).broadcast_to((128, A, NSHF)),
            )
            # DerivErf written e-major contiguous (ACT fast path)
            ybuf = pc.tile([128, NETA * A * NSHF], bf16, tag="ybuf")
            for e in range(NETA):
                nc.scalar.activation(
                    ybuf[:, e * A * NSHF : (e + 1) * A * NSHF],
                    tt_t[:],
                    AF.Derivative_Erf,
                    scale=float(math.sqrt(EtaR[e])),
                )
            # final multiply handles the (e,j,s)->(j,e,s) reorder via
            # strided output (16B runs), contiguous inputs
            yout = pc.tile([128, A * F], bf16, tag="yout")
            yo = yout[:].rearrange("p (j f) -> p j f", f=F)
            for e in range(NETA):
                nc.vector.tensor_tensor(
                    yo[:, :, e * NSHF : (e + 1) * NSHF],
                    ybuf[:, e * A * NSHF : (e + 1) * A * NSHF].rearrange(
                        "p (j s) -> p j s", s=NSHF
                    ),
                    fcm8[:].rearrange("p (j s) -> p j s", s=NSHF),
                    OP.mult,
                )
            nc.sync.dma_start(yt_t.ap()[k], yout[:])

    nc.compile()
    return nc


def _get_nc(EtaR, ShfR):
    key = (
        np.asarray(EtaR, np.float32).tobytes(),
        np.asarray(ShfR, np.float32).tobytes(),
    )
    if key not in _nc_cache:
        _nc_cache[key] = _build(
            np.asarray(EtaR, np.float64), np.asarray(ShfR, np.float64)
        )
    return _nc_cache[key]


def make_in_maps(connectivity, coords, EtaR, ShfR):
    coords = np.asarray(coords, np.float32)
    ShfR = np.asarray(ShfR, np.float32)
    shft_host = np.tile(ShfR, (128, A))
    in_maps = []
    for core in range(N_CORES):
        co = np.ascontiguousarray(coords[core * BPC : (core + 1) * BPC])
        ct_host = np.ascontiguousarray(co.transpose(0, 2, 1)).reshape(BPC, 1, 3 * A)
        xr_host = np.ascontiguousarray(
            np.broadcast_to(ct_host, (BPC, 128, 3 * A))
        )
        in_maps.append({"xr": xr_host, "cp": co, "shft": shft_host})
    return in_maps


def assemble_output(results, connectivity):
    conn = np.asarray(connectivity)
    ys = []
    for core in range(N_CORES):
        # yt[k, p, j*F+f] -> pair (b = k//2, i = 128*(k%2)+p, j)
        tbl = (
            np.asarray(results[core]["yt"])
            .astype(np.float32)
            .reshape(BPC * A * A, F)
        )
        cb = conn[core * BPC : (core + 1) * BPC].astype(np.int64)
        acc, don = cb[..., 0].reshape(EPC), cb[..., 1].reshape(EPC)
        batch = np.repeat(np.arange(BPC, dtype=np.int64), E)
        flat = batch * (A * A) + acc * A + don
        ys.append(tbl[flat])
    return np.concatenate(ys).reshape(B, E, F)


def _ensure_ntff_hook():
    """Provide antenv.axon_hooks (absent in this image) so trace=True works."""
    import types

    try:
        from antenv.axon_hooks import get_axon_ntff_profile_hook  # noqa: F401

        return
    except ImportError:
        pass
    try:
        if "/root/.axon_site" not in sys.path:
            sys.path.insert(0, "/root/.axon_site")
        import antenv
        import trn_agent_boot.trn_boot as _tb

        hook = _tb._ntff_profile_via_ctypes("/opt/axon/libaxon_pjrt.so")
        mod = types.ModuleType("antenv.axon_hooks")
        mod._hook = hook
        mod.get_axon_ntff_profile_hook = lambda: mod._hook
        mod.set_axon_ntff_profile_hook = lambda h: setattr(mod, "_hook", h)
        sys.modules["antenv.axon_hooks"] = mod
        antenv.axon_hooks = mod
    except Exception:
        pass


def kernel(connectivity, coords, EtaR, ShfR, _trace=False):
    from concourse.bass_utils import run_bass_kernel_spmd

    if _trace:
        _ensure_ntff_hook()
    nc = _get_nc(np.asarray(EtaR, np.float64), np.asarray(ShfR, np.float64))
    in_maps = make_in_maps(connectivity, coords, EtaR, ShfR)
    res = run_bass_kernel_spmd(
        nc, in_maps, core_ids=list(range(N_CORES)), trace=_trace
    )
    y = assemble_output(res.results, connectivity)
    if _trace:
        kernel.last_exec_time_ns = res.exec_time_ns
        kernel.last_results = res
    return (np.asarray(connectivity), y)


# revision 30
# speedup vs baseline: 1.2676x; 1.1488x over previous
"""Trainium2 Bass kernel for TorchANI-style radial AEV (gnn_message_passing).

Computation per edge e in batch b:
    d   = || coords[b, acc_e] - coords[b, don_e] ||
    fc  = 0.5*cos(pi*d/Rc) + 0.5         if d <= Rc else 0
    y[b, e, eta*8+shf] = 0.25 * exp(-EtaR[eta]*(d - ShfR[shf])**2) * fc

Strategy (8 NeuronCores, data-parallel over batch, 4 batches/core):
  The output for an edge depends only on its (batch, acc, don) pair, and the
  atom count is tiny (256). Instead of a per-edge gather (no functional
  gather primitive on this stack), each core computes the FULL 256x256
  per-pair feature table for its 4 batches with purely affine data access:
    - pair (i, j) lives at [partition i (mod 128), free j]
    - delta_c = coords[j, c] - coords[i, c] via tensor_scalar with a
      PE-broadcast row operand and a per-partition column scalar
    - d = sqrt(sum delta^2); fc via ACT Sin; features via ACT Exp with
      scale=-eta folding the eta multiply; final multiply by 0.25*fc.
  The table is written as bf16 [262144, 32] (16.8 MB/core, same bytes as the
  edge-mode f32 output). The host resolves y[edge] = table[flat_pair(edge)]
  while unsharding (pure data movement, no arithmetic).
"""

import os
import sys
import math

os.environ.setdefault("MYCRO_LOCAL_CACHE", "1")

for _p in ("/opt/trn_rl_repo", "/root/.axon_site/_ro/trn_rl_repo"):
    if os.path.isdir(_p) and _p not in sys.path:
        sys.path.insert(0, _p)

import numpy as np

RC = 5.2
N_CORES = 8
B, E, A = 32, 32768, 256
BPC = B // N_CORES            # 4 batches per core
EPC = BPC * E                 # 131072 edges per core
NETA, NSHF = 4, 8
F = NETA * NSHF               # 32 features
NK = BPC * 2                  # 8 D-tiles per core: (batch, i-half) [128, 256]

_nc_cache = {}


def _build(EtaR, ShfR):
    from contextlib import ExitStack
    import concourse.tile as tile
    import concourse.mybir as mybir
    from concourse import bacc

    f32 = mybir.dt.float32
    bf16 = mybir.dt.bfloat16
    AF = mybir.ActivationFunctionType
    OP = mybir.AluOpType

    nc = bacc.Bacc(
        "TRN2", target_bir_lowering=False, debug=False, num_devices=N_CORES
    )

    xr_t = nc.dram_tensor("xr", [BPC, 128, 3 * A], f32, kind="ExternalInput")
    cp_t = nc.dram_tensor("cp", [BPC, A, 3], f32, kind="ExternalInput")
    shft_t = nc.dram_tensor("shft", [128, A * NSHF], f32, kind="ExternalInput")
    # y table rows: pair (b, 128*ih+p, j) -> yt[b*2+ih, p, j*F + f]
    yt_t = nc.dram_tensor("yt", [NK, 128, A * F], bf16, kind="ExternalOutput")

    with tile.TileContext(nc) as tc, ExitStack() as ctx:
        consts = ctx.enter_context(tc.tile_pool(name="consts", bufs=1))
        shft_sb = consts.tile([128, A * NSHF], f32)
        nc.sync.dma_start(shft_sb[:], shft_t.ap())
        halfpi = consts.tile([128, 1], f32)
        nc.vector.memset(halfpi[:], math.pi / 2)

        pa = ctx.enter_context(tc.tile_pool(name="pa", bufs=3))
        xrp = ctx.enter_context(tc.tile_pool(name="xr", bufs=BPC))
        dres = ctx.enter_context(tc.tile_pool(name="dres", bufs=NK))
        fres = ctx.enter_context(tc.tile_pool(name="fres", bufs=NK))
        pc = ctx.enter_context(tc.tile_pool(name="pc", bufs=2))

        # ---- Phase A: coords row-broadcast tiles (host-prepared) ----
        xrs = []
        cphs = []
        for b in range(BPC):
            xr = xrp.tile([128, 3 * A], f32, tag="xr")
            nc.sync.dma_start(xr[:], xr_t.ap()[b])
            xrs.append(xr)
            for h in range(2):
                cph = xrp.tile([128, 3], f32, tag="cph")
                nc.sync.dma_start(cph[:], cp_t.ap()[b, 128 * h : 128 * (h + 1), :])
                cphs.append(cph)

        # ---- Phase B: D tiles (squares + sqrt: one ACT set) ----
        dts = []
        for k in range(NK):
            b, cph = k // 2, cphs[k]
            d2 = pa.tile([128, A], f32, tag="d2")
            for c in range(3):
                delta = pa.tile([128, A], f32, tag="delta")
                nc.vector.tensor_scalar(
                    delta[:],
                    xrs[b][:, c * A : (c + 1) * A],
                    cph[:, c : c + 1],
                    None,
                    OP.subtract,
                )
                if c == 0:
                    nc.vector.tensor_mul(d2[:], delta[:], delta[:])
                else:
                    sq = pa.tile([128, A], f32, tag="sq")
                    nc.vector.tensor_mul(sq[:], delta[:], delta[:])
                    nc.vector.tensor_add(d2[:], d2[:], sq[:])
            dt = dres.tile([128, A], f32, tag="dt")
            nc.scalar.sqrt(dt[:], d2[:])
            dts.append(dt)

        # ---- Phase C: fc tiles (Sin set) ----
        fcms = []
        for k in range(NK):
            dc = pa.tile([128, A], f32, tag="dc")
            nc.vector.tensor_scalar(dc[:], dts[k][:], RC, None, OP.min)
            s = pa.tile([128, A], f32, tag="sin")
            nc.scalar.activation(
                s[:], dc[:], AF.Sin, bias=halfpi[:], scale=-math.pi / RC
            )
            fcm = fres.tile([128, A], bf16, tag="fcm")
            # fold 0.25*fc and the Derivative_Erf prefactor sqrt(pi)/2:
            # y = DerivErf(sqrt(eta)*t) * (sqrt(pi)/2) * (0.125*cos + 0.125)
            cc = 0.125 * math.sqrt(math.pi) / 2.0
            nc.vector.tensor_scalar(fcm[:], s[:], cc, cc, OP.mult, OP.add)
            fcms.append(fcm)

        # ---- Phase D: features (erf_derivative set) ----
        for k in range(NK):
            dv = dts[k][:].unsqueeze(2).broadcast_to((128, A, NSHF))
            tt_t = pc.tile([128, A * NSHF], f32, tag="t")
            nc.vector.tensor_tensor(
                tt_t[:].rearrange("p (j s) -> p j s", s=NSHF),
                shft_sb[:].rearrange("p (j s) -> p j s", s=NSHF),
                dv,
                OP.subtract,
            )
            # fcm expanded 8-wide once (then reused by all four eta mults)
            fcm8 = pc.tile([128, A * NSHF], bf16, tag="fcm8")
            nc.vector.tensor_copy(
                fcm8[:].rearrange("p (j s) -> p j s", s=NSHF),
                fcms[k][:].unsqueeze(2).broadcast_to((128, A, NSHF)),
            )
            # DerivErf written e-major contiguous (ACT fast path)
            ybuf = pc.tile([128, NETA * A * NSHF], bf16, tag="ybuf")
            for e in range(NETA):
                nc.scalar.activation(
                    ybuf[:, e * A * NSHF : (e + 1) * A * NSHF],
                    tt_t[:],
                    AF.Derivative_Erf,
                    scale=float(math.sqrt(EtaR[e])),
                )
            # final multiply handles the (e,j,s)->(j,e,s) reorder via
            # strided output (16B runs), contiguous inputs
            yout = pc.tile([128, A * F], bf16, tag="yout")
            yo = yout[:].rearrange("p (j f) -> p j f", f=F)
            for e in range(NETA):
                nc.vector.tensor_tensor(
                    yo[:, :, e * NSHF : (e + 1) * NSHF],
                    ybuf[:, e * A * NSHF : (e + 1) * A * NSHF].rearrange(
                        "p (j s) -> p j s", s=NSHF
                    ),
                    fcm8[:].rearrange("p (j s) -> p j s", s=NSHF),
                    OP.mult,
                )
            nc.sync.dma_start(yt_t.ap()[k], yout[:])

    nc.compile()
    return nc


def _get_nc(EtaR, ShfR):
    key = (
        np.asarray(EtaR, np.float32).tobytes(),
        np.asarray(ShfR, np.float32).tobytes(),
    )
    if key not in _nc_cache:
        _nc_cache[key] = _build(
            np.asarray(EtaR, np.float64), np.asarray(ShfR, np.float64)
        )
    return _nc_cache[key]


def make_in_maps(connectivity, coords, EtaR, ShfR):
    coords = np.asarray(coords, np.float32)
    ShfR = np.asarray(ShfR, np.float32)
    shft_host = np.tile(ShfR, (128, A))
    in_maps = []
    for core in range(N_CORES):
        co = np.ascontiguousarray(coords[core * BPC : (core + 1) * BPC])
        ct_host = np.ascontiguousarray(co.transpose(0, 2, 1)).reshape(BPC, 1, 3 * A)
        xr_host = np.ascontiguousarray(
            np.broadcast_to(ct_host, (BPC, 128, 3 * A))
        )
        in_maps.append({"xr": xr_host, "cp": co, "shft": shft_host})
    return in_maps


def assemble_output(results, connectivity):
    conn = np.asarray(connectivity)
    ys = []
    for core in range(N_CORES):
        # yt[k, p, j*F+f] -> pair (b = k//2, i = 128*(k%2)+p, j)
        tbl = (
            np.asarray(results[core]["yt"])
            .astype(np.float32)
            .reshape(BPC * A * A, F)
        )
        cb = conn[core * BPC : (core + 1) * BPC].astype(np.int64)
        acc, don = cb[..., 0].reshape(EPC), cb[..., 1].reshape(EPC)
        batch = np.repeat(np.arange(BPC, dtype=np.int64), E)
        flat = batch * (A * A) + acc * A + don
        ys.append(tbl[flat])
    return np.concatenate(ys).reshape(B, E, F)


def _ensure_ntff_hook():
    """Provide antenv.axon_hooks (absent in this image) so trace=True works."""
    import types

    try:
        from antenv.axon_hooks import get_axon_ntff_profile_hook  # noqa: F401

        return
    except ImportError:
        pass
    try:
        if "/root/.axon_site" not in sys.path:
            sys.path.insert(0, "/root/.axon_site")
        import antenv
        import trn_agent_boot.trn_boot as _tb

        hook = _tb._ntff_profile_via_ctypes("/opt/axon/libaxon_pjrt.so")
        mod = types.ModuleType("antenv.axon_hooks")
        mod._hook = hook
        mod.get_axon_ntff_profile_hook = lambda: mod._hook
        mod.set_axon_ntff_profile_hook = lambda h: setattr(mod, "_hook", h)
        sys.modules["antenv.axon_hooks"] = mod
        antenv.axon_hooks = mod
    except Exception:
        pass


def kernel(connectivity, coords, EtaR, ShfR, _trace=False):
    from concourse.bass_utils import run_bass_kernel_spmd

    if _trace:
        _ensure_ntff_hook()
    nc = _get_nc(np.asarray(EtaR, np.float64), np.asarray(ShfR, np.float64))
    in_maps = make_in_maps(connectivity, coords, EtaR, ShfR)
    res = run_bass_kernel_spmd(
        nc, in_maps, core_ids=list(range(N_CORES)), trace=_trace
    )
    y = assemble_output(res.results, connectivity)
    if _trace:
        kernel.last_exec_time_ns = res.exec_time_ns
        kernel.last_results = res
    return (np.asarray(connectivity), y)


# revision 31
# speedup vs baseline: 1.5822x; 1.2482x over previous
"""Trainium2 Bass kernel for TorchANI-style radial AEV (gnn_message_passing).

Computation per edge e in batch b:
    d   = || coords[b, acc_e] - coords[b, don_e] ||
    fc  = 0.5*cos(pi*d/Rc) + 0.5         if d <= Rc else 0
    y[b, e, eta*8+shf] = 0.25 * exp(-EtaR[eta]*(d - ShfR[shf])**2) * fc

Strategy (8 NeuronCores, data-parallel over batch, 4 batches/core):
  The output for an edge depends only on its (batch, acc, don) pair, and the
  atom count is tiny (256). Instead of a per-edge gather (no functional
  gather primitive on this stack), each core computes the FULL 256x256
  per-pair feature table for its 4 batches with purely affine data access:
    - pair (i, j) lives at [partition i (mod 128), free j]
    - delta_c = coords[j, c] - coords[i, c] via tensor_scalar with a
      PE-broadcast row operand and a per-partition column scalar
    - d = sqrt(sum delta^2); fc via ACT Sin; features via ACT Exp with
      scale=-eta folding the eta multiply; final multiply by 0.25*fc.
  The table is written as bf16 [262144, 32] (16.8 MB/core, same bytes as the
  edge-mode f32 output). The host resolves y[edge] = table[flat_pair(edge)]
  while unsharding (pure data movement, no arithmetic).
"""

import os
import sys
import math

os.environ.setdefault("MYCRO_LOCAL_CACHE", "1")

for _p in ("/opt/trn_rl_repo", "/root/.axon_site/_ro/trn_rl_repo"):
    if os.path.isdir(_p) and _p not in sys.path:
        sys.path.insert(0, _p)

import numpy as np

RC = 5.2
N_CORES = 8
B, E, A = 32, 32768, 256
BPC = B // N_CORES            # 4 batches per core
EPC = BPC * E                 # 131072 edges per core
NETA, NSHF = 4, 8
F = NETA * NSHF               # 32 features
NK = BPC * 2                  # 8 D-tiles per core: (batch, i-half) [128, 256]

_nc_cache = {}


def _build(EtaR, ShfR):
    from contextlib import ExitStack
    import concourse.tile as tile
    import concourse.mybir as mybir
    from concourse import bacc

    f32 = mybir.dt.float32
    bf16 = mybir.dt.bfloat16
    AF = mybir.ActivationFunctionType
    OP = mybir.AluOpType

    nc = bacc.Bacc(
        "TRN2", target_bir_lowering=False, debug=False, num_devices=N_CORES
    )

    xr_t = nc.dram_tensor("xr", [BPC, 128, 3 * A], f32, kind="ExternalInput")
    cp_t = nc.dram_tensor("cp", [BPC, A, 3], f32, kind="ExternalInput")
    shft_t = nc.dram_tensor("shft", [128, A * NSHF], f32, kind="ExternalInput")
    # y table rows: pair (b, 128*ih+p, j) -> yt[b*2+ih, p, j*F + f]
    yt_t = nc.dram_tensor("yt", [NK, 128, A * F], bf16, kind="ExternalOutput")

    with tile.TileContext(nc) as tc, ExitStack() as ctx:
        consts = ctx.enter_context(tc.tile_pool(name="consts", bufs=1))
        shft_sb = consts.tile([128, A * NSHF], f32)
        nc.sync.dma_start(shft_sb[:], shft_t.ap())
        halfpi = consts.tile([128, 1], f32)
        nc.vector.memset(halfpi[:], math.pi / 2)

        pa = ctx.enter_context(tc.tile_pool(name="pa", bufs=3))
        xrp = ctx.enter_context(tc.tile_pool(name="xr", bufs=BPC))
        dres = ctx.enter_context(tc.tile_pool(name="dres", bufs=NK))
        fres = ctx.enter_context(tc.tile_pool(name="fres", bufs=NK))
        pc = ctx.enter_context(tc.tile_pool(name="pc", bufs=2))

        # ---- Phase A: coords row-broadcast tiles (host-prepared) ----
        xrs = []
        cphs = []
        for b in range(BPC):
            xr = xrp.tile([128, 3 * A], f32, tag="xr")
            nc.sync.dma_start(xr[:], xr_t.ap()[b])
            xrs.append(xr)
            for h in range(2):
                cph = xrp.tile([128, 3], f32, tag="cph")
                nc.sync.dma_start(cph[:], cp_t.ap()[b, 128 * h : 128 * (h + 1), :])
                cphs.append(cph)

        # ---- Phase B: D tiles (squares + sqrt: one ACT set) ----
        dts = []
        for k in range(NK):
            b, cph = k // 2, cphs[k]
            # symmetry: the i>=128 half only needs j>=128 (host uses min/max)
            j0, jw = (128, 128) if k % 2 else (0, A)
            d2 = pa.tile([128, jw], f32, tag=f"d2{k%2}")
            for c in range(3):
                delta = pa.tile([128, jw], f32, tag=f"delta{k%2}")
                nc.vector.tensor_scalar(
                    delta[:],
                    xrs[b][:, c * A + j0 : c * A + j0 + jw],
                    cph[:, c : c + 1],
                    None,
                    OP.subtract,
                )
                if c == 0:
                    nc.vector.tensor_mul(d2[:], delta[:], delta[:])
                else:
                    sq = pa.tile([128, jw], f32, tag=f"sq{k%2}")
                    nc.vector.tensor_mul(sq[:], delta[:], delta[:])
                    nc.vector.tensor_add(d2[:], d2[:], sq[:])
            dt = dres.tile([128, jw], f32, tag=f"dt{k%2}")
            nc.scalar.sqrt(dt[:], d2[:])
            dts.append(dt)

        # ---- Phase C: fc tiles (Sin set) ----
        fcms = []
        for k in range(NK):
            jw = 128 if k % 2 else A
            dc = pa.tile([128, jw], f32, tag=f"dc{k%2}")
            nc.vector.tensor_scalar(dc[:], dts[k][:], RC, None, OP.min)
            s = pa.tile([128, jw], f32, tag=f"sin{k%2}")
            nc.scalar.activation(
                s[:], dc[:], AF.Sin, bias=halfpi[:], scale=-math.pi / RC
            )
            fcm = fres.tile([128, jw], bf16, tag=f"fcm{k%2}")
            # fold 0.25*fc and the Derivative_Erf prefactor sqrt(pi)/2:
            # y = DerivErf(sqrt(eta)*t) * (sqrt(pi)/2) * (0.125*cos + 0.125)
            cc = 0.125 * math.sqrt(math.pi) / 2.0
            nc.vector.tensor_scalar(fcm[:], s[:], cc, cc, OP.mult, OP.add)
            fcms.append(fcm)

        # ---- Phase D: features (erf_derivative set) ----
        for k in range(NK):
            j0, jw = (128, 128) if k % 2 else (0, A)
            dv = dts[k][:].unsqueeze(2).broadcast_to((128, jw, NSHF))
            tt_t = pc.tile([128, jw * NSHF], f32, tag=f"t{k%2}")
            nc.vector.tensor_tensor(
                tt_t[:].rearrange("p (j s) -> p j s", s=NSHF),
                shft_sb[:, j0 * NSHF : (j0 + jw) * NSHF].rearrange(
                    "p (j s) -> p j s", s=NSHF
                ),
                dv,
                OP.subtract,
            )
            # fcm expanded 8-wide once (then reused by all four eta mults)
            fcm8 = pc.tile([128, jw * NSHF], bf16, tag=f"fcm8{k%2}")
            nc.vector.tensor_copy(
                fcm8[:].rearrange("p (j s) -> p j s", s=NSHF),
                fcms[k][:].unsqueeze(2).broadcast_to((128, jw, NSHF)),
            )
            # DerivErf written e-major contiguous (ACT fast path)
            ybuf = pc.tile([128, NETA * jw * NSHF], bf16, tag=f"ybuf{k%2}")
            for e in range(NETA):
                nc.scalar.activation(
                    ybuf[:, e * jw * NSHF : (e + 1) * jw * NSHF],
                    tt_t[:],
                    AF.Derivative_Erf,
                    scale=float(math.sqrt(EtaR[e])),
                )
            # final multiply handles the (e,j,s)->(j,e,s) reorder via
            # strided output (16B runs), contiguous inputs
            yout = pc.tile([128, jw * F], bf16, tag=f"yout{k%2}")
            yo = yout[:].rearrange("p (j f) -> p j f", f=F)
            for e in range(NETA):
                nc.vector.tensor_tensor(
                    yo[:, :, e * NSHF : (e + 1) * NSHF],
                    ybuf[:, e * jw * NSHF : (e + 1) * jw * NSHF].rearrange(
                        "p (j s) -> p j s", s=NSHF
                    ),
                    fcm8[:].rearrange("p (j s) -> p j s", s=NSHF),
                    OP.mult,
                )
            nc.sync.dma_start(
                yt_t.ap()[k, :, j0 * F : (j0 + jw) * F], yout[:]
            )

    nc.compile()
    return nc


def _get_nc(EtaR, ShfR):
    key = (
        np.asarray(EtaR, np.float32).tobytes(),
        np.asarray(ShfR, np.float32).tobytes(),
    )
    if key not in _nc_cache:
        _nc_cache[key] = _build(
            np.asarray(EtaR, np.float64), np.asarray(ShfR, np.float64)
        )
    return _nc_cache[key]


def make_in_maps(connectivity, coords, EtaR, ShfR):
    coords = np.asarray(coords, np.float32)
    ShfR = np.asarray(ShfR, np.float32)
    shft_host = np.tile(ShfR, (128, A))
    in_maps = []
    for core in range(N_CORES):
        co = np.ascontiguousarray(coords[core * BPC : (core + 1) * BPC])
        ct_host = np.ascontiguousarray(co.transpose(0, 2, 1)).reshape(BPC, 1, 3 * A)
        xr_host = np.ascontiguousarray(
            np.broadcast_to(ct_host, (BPC, 128, 3 * A))
        )
        in_maps.append({"xr": xr_host, "cp": co, "shft": shft_host})
    return in_maps


def assemble_output(results, connectivity):
    conn = np.asarray(connectivity)
    ys = []
    for core in range(N_CORES):
        # yt[k, p, j*F+f] -> pair (b = k//2, i = 128*(k%2)+p, j)
        tbl = (
            np.asarray(results[core]["yt"])
            .astype(np.float32)
            .reshape(BPC * A * A, F)
        )
        cb = conn[core * BPC : (core + 1) * BPC].astype(np.int64)
        a0, a1 = cb[..., 0].reshape(EPC), cb[..., 1].reshape(EPC)
        # the device table covers i<=127 full and i>=128 with j>=128;
        # D is symmetric, so look up (min, max)
        acc, don = np.minimum(a0, a1), np.maximum(a0, a1)
        batch = np.repeat(np.arange(BPC, dtype=np.int64), E)
        flat = batch * (A * A) + acc * A + don
        ys.append(tbl[flat])
    return np.concatenate(ys).reshape(B, E, F)


def _ensure_ntff_hook():
    """Provide antenv.axon_hooks (absent in this image) so trace=True works."""
    import types

    try:
        from antenv.axon_hooks import get_axon_ntff_profile_hook  # noqa: F401

        return
    except ImportError:
        pass
    try:
        if "/root/.axon_site" not in sys.path:
            sys.path.insert(0, "/root/.axon_site")
        import antenv
        import trn_agent_boot.trn_boot as _tb

        hook = _tb._ntff_profile_via_ctypes("/opt/axon/libaxon_pjrt.so")
        mod = types.ModuleType("antenv.axon_hooks")
        mod._hook = hook
        mod.get_axon_ntff_profile_hook = lambda: mod._hook
        mod.set_axon_ntff_profile_hook = lambda h: setattr(mod, "_hook", h)
        sys.modules["antenv.axon_hooks"] = mod
        antenv.axon_hooks = mod
    except Exception:
        pass


def kernel(connectivity, coords, EtaR, ShfR, _trace=False):
    from concourse.bass_utils import run_bass_kernel_spmd

    if _trace:
        _ensure_ntff_hook()
    nc = _get_nc(np.asarray(EtaR, np.float64), np.asarray(ShfR, np.float64))
    in_maps = make_in_maps(connectivity, coords, EtaR, ShfR)
    res = run_bass_kernel_spmd(
        nc, in_maps, core_ids=list(range(N_CORES)), trace=_trace
    )
    y = assemble_output(res.results, connectivity)
    if _trace:
        kernel.last_exec_time_ns = res.exec_time_ns
        kernel.last_results = res
    return (np.asarray(connectivity), y)
